# revision 1
# baseline (speedup 1.0000x reference)
"""Trainium2 Bass kernel for nn_CDKANLayer.

Computation (see problem reference):
  w_lag   = softmax(lag_logits, -1)                       [O,I,11]
  window  = x_history[:, T-11:T, :] reversed              [B,11,I]
  x_lagged[b,i,j] = sum_l window[b,l,j] * w_lag[i,j,l]
  xc      = clip(x_lagged, -1, 1)
  y_edge  = sum_c b_splines(xc) * coef                    (cubic B-spline, 8 coefs/edge)
  alpha   = sigmoid(mean_t(x_history)[b,j]*mod_w[i,j] + mod_b[i,j])
  out[b,i]= sum_j y_edge * alpha * sigmoid(adj_logits)[i,j]

Device strategy (8 NeuronCores, SPMD):
  - Shard the in-features axis j = I (128) eight ways: 16 j per core,
    full batch B=256 on every core. Each core returns the partial sum
    over its own j's; the host adds the 8 partials (and transposes).
  - The per-edge spline y_edge(x) is re-parameterized exactly as a
    truncated-power cubic:
        y = c0 + c1 x + c2 x^2 + c3 x^3 + sum_k d_k relu(x - t_k)^3
    with knots t_k = -0.6,-0.2,0.2,0.6 (host precomputes c*/d* per edge
    from `coef` in float64 and folds in the causal mask).
  - Per-core layout: partition dim = i (out-features, 128), free dim =
    (j, b). For a fixed j-slice every per-edge coefficient is a
    per-partition scalar [128,1], so the whole polynomial evaluates with
    fused tensor_scalar / scalar_tensor_tensor ops on the vector engine.
  - TensorE does: T-mean of x_history (ones matmul), x_lagged (K=11
    matmuls per j), the alpha linear term (rank-1 matmuls), and the
    final sum over j (identity-matmul accumulation into PSUM).
  - ScalarE does: Square / 4x Relu features and the sigmoid.
"""

import os
import sys

import ml_dtypes
import numpy as np

for _p in ("/opt/trn_rl_repo", "/root/.axon_site/_ro/trn_rl_repo"):
    if os.path.isdir(_p) and _p not in sys.path:
        sys.path.insert(0, _p)

import concourse.bass as bass  # noqa: E402
import concourse.tile as tile  # noqa: E402
from concourse import bacc, mybir  # noqa: E402
from concourse import bass_utils  # noqa: E402

# ---------------------------------------------------------------- constants
B, T, I, O = 256, 512, 128, 128
L = 11                      # MAX_LAG + 1 lag taps
NCOEF = 8                   # spline coefficients per edge
NCORES = 8
JC = I // NCORES            # j's per core = 16
GRID_SIZE, SPLINE_ORDER = 5, 3
GRID_LO, GRID_HI = -1.0, 1.0
H = (GRID_HI - GRID_LO) / GRID_SIZE
KNOTS = (-0.6, -0.2, 0.2, 0.6)   # interior knots of the clipped domain
NPARAM = 8                  # c0..c3, d1..d4

F32 = mybir.dt.float32
CDT = mybir.dt.bfloat16          # combine-stage dtype
NP_CDT = ml_dtypes.bfloat16
AX = mybir.AxisListType if hasattr(mybir, "AxisListType") else None
ALU = mybir.AluOpType
ACTF = mybir.ActivationFunctionType


# ------------------------------------------------------- host-side spline math
def _b_splines_np(x):
    """float64 copy of the reference b_splines (incl. its 1e-8 epsilons)."""
    g = (np.arange(-SPLINE_ORDER, GRID_SIZE + SPLINE_ORDER + 1, dtype=np.float64)
         * H + GRID_LO)
    x = np.asarray(x, dtype=np.float64)[..., None]
    bases = ((x >= g[:-1]) & (x < g[1:])).astype(np.float64)
    for i in range(1, SPLINE_ORDER + 1):
        t1 = (x - g[: -(i + 1)]) / (g[i:-1] - g[: -(i + 1)] + 1e-8) * bases[..., :-1]
        t2 = (g[i + 1:] - x) / (g[i + 1:] - g[1:-i] + 1e-8) * bases[..., 1:]
        bases = t1 + t2
    return bases


def _segment_poly_mats():
    """A[s] (4x8): on segment s (x in [-1+0.4s, -0.6+0.4s)),
    sum_c coef_c * B_c(x) = sum_d x^d * (A[s][d] @ coef)."""
    mats = []
    for s in range(GRID_SIZE):
        lo = GRID_LO + s * H
        pts = lo + H * np.array([0.125, 0.375, 0.625, 0.875])
        Bm = _b_splines_np(pts)                       # [4, 8]
        V = np.vander(pts, 4, increasing=True)        # [4, 4]
        mats.append(np.linalg.solve(V, Bm))           # [4, 8]
    return np.stack(mats)                             # [5, 4, 8]


def _host_precompute(x_history, coef, lag_logits, mod_w, mod_b, adj_logits):
    """Builds the per-core input dicts. All math in float64 -> float32."""
    xh = np.asarray(x_history, dtype=np.float32)
    coef64 = np.asarray(coef, dtype=np.float64)
    ll = np.asarray(lag_logits, dtype=np.float64)

    # lag softmax  [O, I, L]
    m = ll.max(axis=-1, keepdims=True)
    e = np.exp(ll - m)
    w_lag = (e / e.sum(axis=-1, keepdims=True))       # f64

    # truncated-power parameters per edge, mask folded in
    Amats = _segment_poly_mats()                      # [5, 4, 8]
    a = np.einsum("sdc,oic->sdoi", Amats, coef64)     # [5, 4, O, I]
    mask = 1.0 / (1.0 + np.exp(-np.asarray(adj_logits, dtype=np.float64)[:O, :I]))
    params = np.empty((O, I, NPARAM), dtype=np.float64)
    params[..., 0:4] = np.moveaxis(a[0], 0, -1)       # c0..c3 (segment-0 cubic)
    for k in range(1, 5):                             # d_k = jump of x^3 coef
        params[..., 3 + k] = a[k, 3] - a[k - 1, 3]
    params *= mask[..., None]

    window = xh[:, T - L:T, :][:, ::-1, :]            # [B, L, I] f32

    in_maps = []
    for c in range(NCORES):
        sl = slice(c * JC, (c + 1) * JC)
        # [T, B, JC] so the DMA inner runs are 16 KiB contiguous
        xh_t = np.ascontiguousarray(xh[:, :, sl].transpose(1, 0, 2))
        win = np.ascontiguousarray(window[:, :, sl].transpose(1, 2, 0))  # [L, JC, B]
        wlg = np.ascontiguousarray(
            w_lag[:, sl, :].transpose(2, 1, 0)).astype(np.float32)       # [L, JC, O]
        ct = np.ascontiguousarray(
            params[:, sl, :].transpose(0, 2, 1)).astype(np.float32)      # [O, NPARAM, JC]
        mwT = np.ascontiguousarray(
            np.asarray(mod_w, np.float32)[:, sl].T)[None]                # [1, JC, O]
        mbC = np.ascontiguousarray(
            np.asarray(mod_b, np.float32)[:, sl])                        # [O, JC]
        in_maps.append({
            "xh_t": xh_t,
            "win": win.astype(np.float32),
            "wlag": wlg,
            "ct": ct,
            "mwT": mwT,
            "mbC": mbC,
            "ones_mean": np.full((128, 1), 1.0 / T, dtype=np.float32),
            "ones_b": np.ones((1, B), dtype=np.float32),
            "ident": np.eye(128, dtype=np.float32),
            "nknots": np.tile(-np.asarray(KNOTS, np.float32), (128, 1)),
        })
    return in_maps


# ------------------------------------------------------------- device program
def _build_program():
    nc = bacc.Bacc("TRN2", target_bir_lowering=False, debug=False,
                   num_devices=NCORES)

    xh_t = nc.dram_tensor("xh_t", [T, B, JC], F32, kind="ExternalInput")
    win = nc.dram_tensor("win", [L, JC, B], F32, kind="ExternalInput")
    wlag = nc.dram_tensor("wlag", [L, JC, O], F32, kind="ExternalInput")
    ct = nc.dram_tensor("ct", [O, NPARAM, JC], F32, kind="ExternalInput")
    mwT = nc.dram_tensor("mwT", [1, JC, O], F32, kind="ExternalInput")
    mbC = nc.dram_tensor("mbC", [O, JC], F32, kind="ExternalInput")
    ones_mean = nc.dram_tensor("ones_mean", [128, 1], F32, kind="ExternalInput")
    ones_b = nc.dram_tensor("ones_b", [1, B], F32, kind="ExternalInput")
    ident = nc.dram_tensor("ident", [128, 128], F32, kind="ExternalInput")
    nknots = nc.dram_tensor("nknots", [128, 4], F32, kind="ExternalInput")
    out_d = nc.dram_tensor("outp", [O, B], F32, kind="ExternalOutput")

    TCH = T // 128            # 4 time chunks for the mean
    HCOLS = (B * JC) // 4     # quarter-chunk free size (1024)

    with tile.TileContext(nc) as tc:
        with (
            tc.tile_pool(name="pers", bufs=1) as pers,  # persistent SBUF
            tc.tile_pool(name="xh", bufs=3) as xhp,     # streamed history
            tc.tile_pool(name="scr", bufs=2) as scr,    # per-slice scratch
            tc.tile_pool(name="pwork", bufs=4,
                         space=bass.MemorySpace.PSUM) as pwork,
            tc.tile_pool(name="pmean", bufs=3,
                         space=bass.MemorySpace.PSUM) as pmean,
            tc.tile_pool(name="pout", bufs=1,
                         space=bass.MemorySpace.PSUM) as pout,
        ):
            # ---- persistent loads (small, start immediately)
            early = tc.tile_pool(name="early", bufs=1)
            epool = early.__enter__()
            win_sb = epool.tile([L, JC, B], F32, tag="win", name="win_sb")
            nc.sync.dma_start(win_sb[:], win[:])
            wlag_sb = epool.tile([L, JC, O], F32, tag="wlag", name="wlag_sb")
            nc.sync.dma_start(wlag_sb[:], wlag[:])
            ct_sb = pers.tile([O, NPARAM, JC], F32, tag="ct")
            nc.sync.dma_start(ct_sb[:], ct[:])
            onesm_sb = pers.tile([128, 1], F32, tag="onesm")
            nc.sync.dma_start(onesm_sb[:], ones_mean[:])
            ident_sb = pers.tile([128, 128], F32, tag="ident")
            nc.sync.dma_start(ident_sb[:], ident[:])
            nknots_sb = pers.tile([128, 4], F32, tag="nknots")
            nc.sync.dma_start(nknots_sb[:], nknots[:])

            # row-packed [33, 6400]: row0 = xm (0:4096) + mwT (4096:6144);
            # row32 = mbT (0:2048) + ones_b (2048:2304). Keeps lhsT/rhs of
            # each rank-1 matmul on the same base partition.
            rows_sb = pers.tile([1, 6144], F32, tag="rows")
            nc.sync.dma_start(rows_sb[0:1, 4096:6144],
                              mwT[:].rearrange("p j o -> p (j o)"))
            mbC_sb = pers.tile([O, JC], F32, tag="mbC")
            nc.sync.dma_start(mbC_sb[:], mbC[:])

            # ---- x_lagged: per-j K=11 matmuls -> PSUM [i, b]
            ps_xl = [pwork.tile([128, 512], F32, tag="wk", name=f"ps_xl{t}")
                     for t in range(8)]
            for jl in range(JC):
                nc.tensor.matmul(
                    ps_xl[jl // 2][:, (jl % 2) * B:(jl % 2) * B + B],
                    wlag_sb[:, jl, :], win_sb[:, jl, :],
                    start=True, stop=True)

            early.__exit__(None, None, None)   # win/wlag zone -> scr reuse
            scr_cm = tc.tile_pool(name="scr", bufs=3)
            scr = scr_cm.__enter__()

            # ---- clip -> xc f32 [i, (j, b)] in SBUF
            xc = pers.tile([128, JC, B], F32, tag="xc")
            for jl in range(JC):
                nc.vector.tensor_scalar(
                    xc[:, jl, :],
                    ps_xl[jl // 2][:, (jl % 2) * B:(jl % 2) * B + B],
                    -1.0, 1.0, op0=ALU.max, op1=ALU.min)

            # ---- spline features (f32: the truncated-power combine cancels
            # heavily, so q_k / x2 must keep full precision)
            x2 = pers.tile([128, JC, B], F32, tag="x2")
            nc.scalar.activation(x2[:], xc[:], ACTF.Square)
            x3 = pers.tile([128, JC, B], F32, tag="x3")
            nc.gpsimd.tensor_tensor(x3[:], xc[:], x2[:], op=ALU.mult)
            rq = []
            for k, t_k in enumerate(KNOTS):
                r = pers.tile([128, JC, B], F32, tag=f"r{k}", name=f"r{k}")
                nc.scalar.activation(r[:], xc[:], ACTF.Relu,
                                     bias=nknots_sb[:, k:k + 1])
                nc.scalar.activation(r[:], r[:], ACTF.Square)   # q_k in place
                # r3_k = (xc - t_k) * q_k, in place over q_k
                nc.vector.scalar_tensor_tensor(
                    r[:], xc[:], -t_k, r[:], op0=ALU.add, op1=ALU.mult)
                rq.append(r)

            # ---- streamed mean over T (TensorE ones-matmul)
            ps_mean = [pmean.tile([65, 512], F32, tag="mean", name=f"pm{t}")
                       for t in range(3)]

            def _mrow(nb):
                return ps_mean[nb // 3][(nb % 3) * 32:(nb % 3) * 32 + 1, :]

            for chk in range(TCH):
                for hf in range(4):
                    xt = xhp.tile([128, HCOLS], F32, tag="xh")
                    nc.sync.dma_start(
                        xt[:],
                        xh_t[chk * 128:(chk + 1) * 128, :, :]
                        .rearrange("t b j -> t (b j)")
                        [:, hf * HCOLS:(hf + 1) * HCOLS])
                    for nb in range(2):
                        nc.tensor.matmul(
                            _mrow(hf * 2 + nb), onesm_sb[:],
                            xt[:, nb * 512:(nb + 1) * 512],
                            start=(chk == 0), stop=(chk == TCH - 1))
            for nb in range(8):
                nc.scalar.activation(
                    rows_sb[0:1, nb * 512:(nb + 1) * 512], _mrow(nb),
                    ACTF.Copy)

            # ---- alpha: rank-1 matmuls + sigmoid -> bf16 SBUF
            xm_bj = rows_sb[0:1, 0:4096].rearrange("p (b j) -> p j b", j=JC)
            ps_lin = [pwork.tile([128, 512], F32, tag="wk", name=f"ps_lin{t}")
                      for t in range(8)]
            alpha = pers.tile([128, JC, B], F32, tag="alpha")
            for jl in range(JC):
                dst = ps_lin[jl // 2][:, (jl % 2) * B:(jl % 2) * B + B]
                nc.tensor.matmul(dst,
                                 rows_sb[0:1, 4096 + jl * O:4096 + (jl + 1) * O],
                                 xm_bj[:, jl, :], start=True, stop=True)
                nc.scalar.activation(alpha[:, jl, :], dst, ACTF.Sigmoid,
                                     bias=mbC_sb[:, jl:jl + 1])

            # ---- per-j-slice spline evaluation (f32) -> yfull (bf16)
            yfull = pers.tile([128, JC, B], F32, tag="yfull")
            for jl in range(JC):
                xc_j = xc[:, jl, :]
                x2_j = x2[:, jl, :]
                c0 = ct_sb[:, 0, jl:jl + 1]
                c1 = ct_sb[:, 1, jl:jl + 1]
                c2 = ct_sb[:, 2, jl:jl + 1]
                c3 = ct_sb[:, 3, jl:jl + 1]

                x3_j = x3[:, jl, :]
                h1 = scr.tile([128, B], F32, tag="h1")
                nc.vector.tensor_scalar(h1[:], xc_j, c1, c0,
                                        op0=ALU.mult, op1=ALU.add)
                h2 = scr.tile([128, B], F32, tag="h2")
                nc.vector.scalar_tensor_tensor(
                    h2[:], x2_j, c2, h1[:], op0=ALU.mult, op1=ALU.add)
                y = scr.tile([128, B], F32, tag="y")
                nc.vector.scalar_tensor_tensor(
                    y[:], x3_j, c3, h2[:], op0=ALU.mult, op1=ALU.add)
                for k in range(len(KNOTS)):
                    dk = ct_sb[:, 4 + k, jl:jl + 1]
                    ydst = (yfull[:, jl, :] if k == len(KNOTS) - 1
                            else scr.tile([128, B], F32, tag="y", name="y2"))
                    nc.vector.scalar_tensor_tensor(
                        ydst, rq[k][:, jl, :], dk, y[:],
                        op0=ALU.mult, op1=ALU.add)
                    y = ydst

            # ---- z = y * alpha (bf16 2x), summed over j via identity matmuls
            ps_out = pout.tile([128, B], F32, tag="out")
            for jl in range(JC):
                z = scr.tile([128, B], F32, tag="z")
                nc.gpsimd.tensor_tensor(z[:], yfull[:, jl, :],
                                        alpha[:, jl, :], op=ALU.mult)
                nc.tensor.matmul(ps_out[:], ident_sb[:], z[:],
                                 start=(jl == 0), stop=(jl == JC - 1))

            out_sb = scr.tile([128, B], F32, tag="z", name="out_sb")
            nc.vector.tensor_copy(out_sb[:], ps_out[:])
            nc.sync.dma_start(out_d[:], out_sb[:])
            scr_cm.__exit__(None, None, None)

    nc.compile()
    return nc


_CACHED_NC = None


def _get_program():
    global _CACHED_NC
    if _CACHED_NC is None:
        _CACHED_NC = _build_program()
    return _CACHED_NC


# ------------------------------------------------------------------ entry
def kernel(x_history, coef, lag_logits, mod_w, mod_b, adj_logits):
    in_maps = _host_precompute(x_history, coef, lag_logits, mod_w, mod_b,
                               adj_logits)
    nc = _get_program()
    res = bass_utils.run_bass_kernel_spmd(nc, in_maps,
                                          core_ids=list(range(NCORES)))
    total = np.zeros((O, B), dtype=np.float64)
    for c in range(NCORES):
        total += np.asarray(res.results[c]["outp"], dtype=np.float64)
    return np.ascontiguousarray(total.T.astype(np.float32))


# -------------------------------------------- pure-numpy emulation (testing)
def emulate(x_history, coef, lag_logits, mod_w, mod_b, adj_logits):
    """Numpy mirror of the device algorithm (same math, no hardware)."""
    in_maps = _host_precompute(x_history, coef, lag_logits, mod_w, mod_b,
                               adj_logits)
    total = np.zeros((O, B), dtype=np.float64)
    for c in range(NCORES):
        im = in_maps[c]
        part = emulate_core(im)
        total += part
    return total.T.astype(np.float32)


def emulate_core(im):
    """One core's partial output [O, B] in float32-ish numpy."""
    xh_t = im["xh_t"].astype(np.float32)          # [T, B, JC]
    win = im["win"].astype(np.float32)            # [L, JC, B]
    wlg = im["wlag"].astype(np.float32)           # [L, JC, O]
    ctp = im["ct"].astype(np.float32)             # [O, NPARAM, JC]
    mwT = im["mwT"][0].astype(np.float32)         # [JC, O]
    mbT = im["mbC"].astype(np.float32).T          # [JC, O]

    xm = xh_t.mean(axis=0)                        # [B, JC]
    part = np.zeros((O, B), dtype=np.float32)
    for jl in range(JC):
        xl = wlg[:, jl, :].T @ win[:, jl, :]      # [O, B]
        xc = np.clip(xl, -1.0, 1.0)
        x2 = xc * xc
        c = ctp[:, :, jl]                         # [O, NPARAM]
        y = (c[:, 0:1] + c[:, 2:3] * x2) + xc * (c[:, 1:2] + c[:, 3:4] * x2)
        for k, t_k in enumerate(KNOTS):
            r = np.maximum(xc - t_k, 0.0)
            y = y + c[:, 4 + k:5 + k] * (r * r) * (xc - t_k)
        lin = mwT[jl][:, None] * xm[:, jl][None, :] + mbT[jl][:, None]
        alpha = 1.0 / (1.0 + np.exp(-lin))        # [O, B]
        part += y * alpha
    return part.astype(np.float64)



# revision 2
# speedup vs baseline: 1.7025x; 1.7025x over previous
"""Trainium2 Bass kernel for nn_CDKANLayer (v2).

Computation (see problem reference):
  w_lag   = softmax(lag_logits, -1)                       [O,I,11]
  window  = x_history[:, T-11:T, :] reversed              [B,11,I]
  x_lagged[b,i,j] = sum_l window[b,l,j] * w_lag[i,j,l]
  xc      = clip(x_lagged, -1, 1)
  y_edge  = sum_c b_splines(xc) * coef                    (cubic B-spline)
  alpha   = sigmoid(mean_t(x_history)[b,j]*mod_w[i,j] + mod_b[i,j])
  out[b,i]= sum_j y_edge * alpha * sigmoid(adj_logits)[i,j]

v2 design (8 cores, shard in-features j; each core: 16 j x full B=256):
  - Spline re-parameterized as a TWO-SIDED truncated-power cubic around
    the center segment (kills the cancellation that forced fp32):
      y = c0 + c1 x + c2 x^2 + c3 x^3
        + dR1 relu(x-.2)^3 + dR2 relu(x-.6)^3
        + dL1 relu(-x-.2)^3 + dL2 relu(-x-.6)^3
    (host precomputes all 8 per-edge coefs in float64, mask folded in;
    measured end-to-end rel err ~2e-3 vs the 2e-2 gate).
  - Features (xc, x^2, x^3, 4x relu^3) computed full-size in fp16:
    tensor_scalar runs 4x and tensor_tensor 2x on DVE for fp16.
  - The per-edge combine sum_p c_p[i,j]*f_p[i,j,b] runs on the TENSOR
    engine as 8 accumulating block-diagonal fp16 matmuls per j:
    lhsT = diag(c_p[:,j]) so out[i,b] += c_p[i]*f_p[i,b]. The 128
    diagonal tiles are host-built and DMAed (4 MB, overlapped).
  - mean_t(x_history) streams as fp8 through K=128 matmuls with an
    all-ones lhsT [128,128] -> PSUM rows already broadcast across all
    partitions; ACT applies sigmoid straight from PSUM with per-edge
    scale=mod_w/T and bias=mod_b.
  - x_lagged: 16 bf16 matmuls (K=11).  z = y*alpha on DVE (f32 PSUM x
    f32 alpha -> fp16), summed over j on GpSimd; fp16 partials to host.
"""

import os
import sys

import ml_dtypes
import numpy as np

for _p in ("/opt/trn_rl_repo", "/root/.axon_site/_ro/trn_rl_repo"):
    if os.path.isdir(_p) and _p not in sys.path:
        sys.path.insert(0, _p)

import concourse.bass as bass  # noqa: E402
import concourse.tile as tile  # noqa: E402
from concourse import bacc, mybir  # noqa: E402
from concourse import bass_utils  # noqa: E402

# ---------------------------------------------------------------- constants
B, T, I, O = 256, 512, 128, 128
L = 11                      # MAX_LAG + 1 lag taps
NCORES = 8
JC = I // NCORES            # j's per core = 16
GRID_SIZE, SPLINE_ORDER = 5, 3
GRID_LO, GRID_HI = -1.0, 1.0
H = (GRID_HI - GRID_LO) / GRID_SIZE
NP = 8                      # features: 1, x, x2, x3, rR1^3, rR2^3, rL1^3, rL2^3

F32 = mybir.dt.float32
F16 = mybir.dt.float16
BF16 = mybir.dt.bfloat16
FP8 = mybir.dt.float8e4
ALU = mybir.AluOpType
ACTF = mybir.ActivationFunctionType

NP_F16 = np.float16
NP_BF16 = ml_dtypes.bfloat16
NP_FP8 = ml_dtypes.float8_e4m3


# ------------------------------------------------------- host-side spline math
def _b_splines_np(x):
    """float64 copy of the reference b_splines (incl. its 1e-8 epsilons)."""
    g = (np.arange(-SPLINE_ORDER, GRID_SIZE + SPLINE_ORDER + 1, dtype=np.float64)
         * H + GRID_LO)
    x = np.asarray(x, dtype=np.float64)[..., None]
    bases = ((x >= g[:-1]) & (x < g[1:])).astype(np.float64)
    for i in range(1, SPLINE_ORDER + 1):
        t1 = (x - g[: -(i + 1)]) / (g[i:-1] - g[: -(i + 1)] + 1e-8) * bases[..., :-1]
        t2 = (g[i + 1:] - x) / (g[i + 1:] - g[1:-i] + 1e-8) * bases[..., 1:]
        bases = t1 + t2
    return bases


def _segment_poly_mats():
    """A[s] (4x8): on segment s, sum_c coef_c*B_c(x) = sum_d x^d*(A[s][d]@coef)."""
    mats = []
    for s in range(GRID_SIZE):
        lo = GRID_LO + s * H
        pts = lo + H * np.array([0.125, 0.375, 0.625, 0.875])
        Bm = _b_splines_np(pts)                       # [4, 8]
        V = np.vander(pts, 4, increasing=True)        # [4, 4]
        mats.append(np.linalg.solve(V, Bm))           # [4, 8]
    return np.stack(mats)                             # [5, 4, 8]


def _two_sided_params(coef64, mask):
    """[O, I, 8] float64: c0..c3 (center-segment cubic), dR1,dR2,dL1,dL2."""
    Am = _segment_poly_mats()                          # [5,4,8]
    a = np.einsum("sdc,oic->sdoi", Am, coef64)         # [5,4,O,I]
    p = np.empty((O, I, NP), dtype=np.float64)
    p[..., 0:4] = np.moveaxis(a[2], 0, -1)             # center cubic
    p[..., 4] = a[3, 3] - a[2, 3]                      # jump at +0.2
    p[..., 5] = a[4, 3] - a[3, 3]                      # jump at +0.6
    p[..., 6] = -(a[1, 3] - a[2, 3])                   # knot -0.2, relu(-x-.2)^3
    p[..., 7] = -(a[0, 3] - a[1, 3])                   # knot -0.6, relu(-x-.6)^3
    return p * mask[..., None]


def _host_precompute(x_history, coef, lag_logits, mod_w, mod_b, adj_logits):
    xh = np.asarray(x_history, dtype=np.float32)
    coef64 = np.asarray(coef, dtype=np.float64)
    ll = np.asarray(lag_logits, dtype=np.float64)

    m = ll.max(axis=-1, keepdims=True)
    e = np.exp(ll - m)
    w_lag = e / e.sum(axis=-1, keepdims=True)          # [O,I,L] f64

    mask = 1.0 / (1.0 + np.exp(-np.asarray(adj_logits, np.float64)[:O, :I]))
    params = _two_sided_params(coef64, mask)           # [O,I,8]

    window = xh[:, T - L:T, :][:, ::-1, :]             # [B,L,I]
    xh_tjb = np.ascontiguousarray(xh.transpose(1, 2, 0))  # [T, I, B]
    xh8_full = xh_tjb.astype(NP_FP8)

    rng = np.arange(128)
    in_maps = []
    for c in range(NCORES):
        sl = slice(c * JC, (c + 1) * JC)
        win = np.ascontiguousarray(
            window[:, :, sl].transpose(1, 2, 0)).astype(NP_BF16)   # [L,JC,B]
        wlg = np.ascontiguousarray(
            w_lag[:, sl, :].transpose(2, 1, 0)).astype(NP_BF16)    # [L,JC,O]
        xh8 = np.ascontiguousarray(xh8_full[:, sl, :])             # [T,JC,B]
        # diagonal combine tiles: [128 rows, p, j, 128 cols]
        dg = np.zeros((128, NP, JC, 128), dtype=NP_F16)
        dg[rng, :, :, rng] = params[:, sl, :].transpose(0, 2, 1)   # [O,NP,JC]
        sigsc = np.ascontiguousarray(
            np.asarray(mod_w, np.float64)[:, sl] / T).astype(np.float32)
        sigbi = np.ascontiguousarray(
            np.asarray(mod_b, np.float64)[:, sl]).astype(np.float32)
        in_maps.append({
            "win": win,
            "wlag": wlg,
            "xh8": xh8,
            "diag": np.ascontiguousarray(dg.reshape(128, NP * JC * 128)),
            "ones16": np.ones((128, B), dtype=NP_F16),
            "ones8": np.ones((128, 128), dtype=NP_FP8),
            "sigsc": sigsc,
            "sigbi": sigbi,
        })
    return in_maps


# ------------------------------------------------------------- device program
def _build_program():
    nc = bacc.Bacc("TRN2", target_bir_lowering=False, debug=False,
                   num_devices=NCORES)

    win_d = nc.dram_tensor("win", [L, JC, B], BF16, kind="ExternalInput")
    wlag_d = nc.dram_tensor("wlag", [L, JC, O], BF16, kind="ExternalInput")
    xh8_d = nc.dram_tensor("xh8", [T, JC, B], FP8, kind="ExternalInput")
    diag_d = nc.dram_tensor("diag", [128, NP * JC * 128], F16,
                            kind="ExternalInput")
    ones16_d = nc.dram_tensor("ones16", [128, B], F16, kind="ExternalInput")
    ones8_d = nc.dram_tensor("ones8", [128, 128], FP8, kind="ExternalInput")
    sigsc_d = nc.dram_tensor("sigsc", [O, JC], F32, kind="ExternalInput")
    sigbi_d = nc.dram_tensor("sigbi", [O, JC], F32, kind="ExternalInput")
    out_d = nc.dram_tensor("outp", [O, B], F16, kind="ExternalOutput")

    KN = [(0, -0.2), (0, -0.6), (1, -0.2), (1, -0.6)]   # (use_negx, bias)

    with tile.TileContext(nc) as tc:
        with (
            tc.tile_pool(name="pers", bufs=1) as pers,
            tc.tile_pool(name="xhp", bufs=3) as xhp,
            tc.tile_pool(name="pxl", bufs=4, space=bass.MemorySpace.PSUM) as pxl,
            tc.tile_pool(name="pmean", bufs=2, space=bass.MemorySpace.PSUM) as pmean,
            tc.tile_pool(name="py", bufs=2, space=bass.MemorySpace.PSUM) as py,
        ):
            # ---------------- persistent loads
            win_sb = pers.tile([L, JC, B], BF16, tag="win")
            nc.sync.dma_start(win_sb[:], win_d[:])
            wlag_sb = pers.tile([L, JC, O], BF16, tag="wlag")
            nc.sync.dma_start(wlag_sb[:], wlag_d[:])
            ones16 = pers.tile([128, B], F16, tag="ones16")
            nc.sync.dma_start(ones16[:], ones16_d[:])
            ones8 = pers.tile([128, 128], FP8, tag="ones8")
            nc.sync.dma_start(ones8[:], ones8_d[:])
            sigsc = pers.tile([O, JC], F32, tag="sigsc")
            nc.sync.dma_start(sigsc[:], sigsc_d[:])
            sigbi = pers.tile([O, JC], F32, tag="sigbi")
            nc.sync.dma_start(sigbi[:], sigbi_d[:])
            diag = pers.tile([128, NP * JC * 128], F16, tag="diag")
            DGCH = NP * JC * 128 // 8
            for i in range(8):
                nc.sync.dma_start(diag[:, i * DGCH:(i + 1) * DGCH],
                                  diag_d[:, i * DGCH:(i + 1) * DGCH])

            def dg(p, j):
                off = (p * JC + j) * 128
                return diag[:, off:off + 128]

            # ---------------- x_lagged matmuls (bf16, K=11) -> 8 psum tiles
            ps_xl = []
            for t in range(8):                      # tile t covers j = 2t, 2t+1
                pt = pxl.tile([128, 2 * B], F32, tag="xl", name=f"xl{t}")
                ps_xl.append(pt)
                for h in range(2):
                    jl = 2 * t + h
                    nc.tensor.matmul(pt[:, h * B:(h + 1) * B],
                                     wlag_sb[:, jl, :], win_sb[:, jl, :],
                                     start=True, stop=True)

            # ---------------- clip -> xc fp16 (1x from PSUM)
            xc = pers.tile([128, JC * B], F16, tag="xc")
            for t in range(8):
                nc.vector.tensor_scalar(xc[:, t * 512:(t + 1) * 512],
                                        ps_xl[t][:], -1.0, 1.0,
                                        op0=ALU.max, op1=ALU.min)

            # ---------------- features (fp16)
            negx = pers.tile([128, JC * B], F16, tag="negx")
            nc.vector.tensor_scalar(negx[:], xc[:], -1.0, None, op0=ALU.mult)
            r_t = []
            for k, (useneg, bia) in enumerate(KN):
                r = pers.tile([128, JC * B], F16, tag=f"r{k}", name=f"r{k}")
                nc.vector.tensor_scalar(r[:], (negx if useneg else xc)[:],
                                        bia, 0.0, op0=ALU.add, op1=ALU.max)
                r_t.append(r)
            # squares: knots 0,2 on ACT; knots 1,3 on DVE; x2 on ACT
            x2 = pers.tile([128, JC * B], F16, tag="x2")
            nc.scalar.activation(x2[:], xc[:], ACTF.Square)
            q_t = []
            for k in range(4):
                q = pers.tile([128, JC * B], F16, tag=f"q{k}", name=f"q{k}")
                if k in (0, 2):
                    nc.scalar.activation(q[:], r_t[k][:], ACTF.Square)
                else:
                    nc.vector.tensor_tensor(q[:], r_t[k][:], r_t[k][:],
                                            op=ALU.mult)
                q_t.append(q)
            x3 = pers.tile([128, JC * B], F16, tag="x3")
            nc.vector.tensor_tensor(x3[:], x2[:], xc[:], op=ALU.mult)
            r3_t = []
            for k in range(4):
                r3 = pers.tile([128, JC * B], F16, tag=f"c{k}", name=f"c{k}")
                nc.vector.tensor_tensor(r3[:], q_t[k][:], r_t[k][:], op=ALU.mult)
                r3_t.append(r3)

            # ---------------- mean stream: fp8 matmuls, rows broadcast
            # bank r covers j = 2r, 2r+1; 4 t-chunk matmuls accumulate
            alpha = pers.tile([128, JC * B], F32, tag="alpha")
            for r in range(8):
                pm = pmean.tile([128, 512], F32, tag="mean", name=f"pm{r}")
                for ch in range(4):
                    xt = xhp.tile([128, 512], FP8, tag="xh")
                    nc.sync.dma_start(
                        xt[:],
                        xh8_d[ch * 128:(ch + 1) * 128, :, :]
                        .rearrange("t j b -> t (j b)")[:, r * 512:(r + 1) * 512])
                    nc.tensor.matmul(pm[:], ones8[:], xt[:],
                                     start=(ch == 0), stop=(ch == 3))
                for h in range(2):
                    jl = 2 * r + h
                    nc.scalar.activation(
                        alpha[:, jl * B:(jl + 1) * B], pm[:, h * B:(h + 1) * B],
                        ACTF.Sigmoid, bias=sigbi[:, jl:jl + 1],
                        scale=sigsc[:, jl:jl + 1])

            # ---------------- per-j combine on TensorE + z + j-sum
            feats = [None, xc, x2, x3] + r3_t
            zb = pers.tile([128, JC * B], F16, tag="zb")
            acc = pers.tile([128, B], F16, tag="acc")
            for jl in range(JC):
                pyt = py.tile([128, B], F32, tag="y", name=f"y{jl}")
                for p in range(NP):
                    rhs = (ones16[:] if p == 0
                           else feats[p][:, jl * B:(jl + 1) * B])
                    nc.tensor.matmul(pyt[:], dg(p, jl), rhs,
                                     start=(p == 0), stop=(p == NP - 1))
                nc.vector.tensor_tensor(zb[:, jl * B:(jl + 1) * B], pyt[:],
                                        alpha[:, jl * B:(jl + 1) * B],
                                        op=ALU.mult)
                if jl == 1:
                    nc.gpsimd.tensor_tensor(acc[:], zb[:, 0:B], zb[:, B:2 * B],
                                            op=ALU.add)
                elif jl > 1:
                    nc.gpsimd.tensor_tensor(acc[:], acc[:],
                                            zb[:, jl * B:(jl + 1) * B],
                                            op=ALU.add)

            nc.sync.dma_start(out_d[:], acc[:])

    nc.compile()
    return nc


_CACHED_NC = None


def _get_program():
    global _CACHED_NC
    if _CACHED_NC is None:
        _CACHED_NC = _build_program()
    return _CACHED_NC


# ------------------------------------------------------------------ entry
def kernel(x_history, coef, lag_logits, mod_w, mod_b, adj_logits):
    in_maps = _host_precompute(x_history, coef, lag_logits, mod_w, mod_b,
                               adj_logits)
    nc = _get_program()
    res = bass_utils.run_bass_kernel_spmd(nc, in_maps,
                                          core_ids=list(range(NCORES)))
    total = np.zeros((O, B), dtype=np.float64)
    for c in range(NCORES):
        total += np.asarray(res.results[c]["outp"], dtype=np.float64)
    return np.ascontiguousarray(total.T.astype(np.float32))


# -------------------------------------------- pure-numpy emulation (testing)
def emulate(x_history, coef, lag_logits, mod_w, mod_b, adj_logits):
    """Numpy mirror of the v2 device algorithm (f32-ish, no dtype sim)."""
    in_maps = _host_precompute(x_history, coef, lag_logits, mod_w, mod_b,
                               adj_logits)
    total = np.zeros((O, B), dtype=np.float64)
    for c in range(NCORES):
        total += emulate_core(in_maps[c])
    return total.T.astype(np.float32)


def emulate_core(im):
    win = im["win"].astype(np.float64)            # [L,JC,B]
    wlg = im["wlag"].astype(np.float64)           # [L,JC,O]
    dgf = im["diag"].astype(np.float64).reshape(128, NP, JC, 128)
    params = dgf[np.arange(128), :, :, np.arange(128)]   # [128,NP,JC] (o,p,j)
    xm = im["xh8"].astype(np.float64).mean(axis=0)       # [JC,B]
    sigsc = im["sigsc"].astype(np.float64)        # [O,JC]
    sigbi = im["sigbi"].astype(np.float64)

    part = np.zeros((O, B), dtype=np.float64)
    for jl in range(JC):
        xl = wlg[:, jl, :].T @ win[:, jl, :]      # [O,B]
        x = np.clip(xl, -1.0, 1.0)
        f = [np.ones_like(x), x, x * x, x ** 3,
             np.maximum(x - 0.2, 0) ** 3, np.maximum(x - 0.6, 0) ** 3,
             np.maximum(-x - 0.2, 0) ** 3, np.maximum(-x - 0.6, 0) ** 3]
        y = np.zeros_like(x)
        for p in range(NP):
            y += params[:, p, jl][:, None] * f[p]
        lin = sigsc[:, jl][:, None] * (xm[jl] * T)[None, :] + sigbi[:, jl][:, None]
        part += y / (1.0 + np.exp(-lin))
    return part


# revision 7
# speedup vs baseline: 2.0049x; 1.1776x over previous
"""Trainium2 Bass kernel for nn_CDKANLayer (v2).

Computation (see problem reference):
  w_lag   = softmax(lag_logits, -1)                       [O,I,11]
  window  = x_history[:, T-11:T, :] reversed              [B,11,I]
  x_lagged[b,i,j] = sum_l window[b,l,j] * w_lag[i,j,l]
  xc      = clip(x_lagged, -1, 1)
  y_edge  = sum_c b_splines(xc) * coef                    (cubic B-spline)
  alpha   = sigmoid(mean_t(x_history)[b,j]*mod_w[i,j] + mod_b[i,j])
  out[b,i]= sum_j y_edge * alpha * sigmoid(adj_logits)[i,j]

v2 design (8 cores, shard in-features j; each core: 16 j x full B=256):
  - Spline re-parameterized as a TWO-SIDED truncated-power cubic around
    the center segment (kills the cancellation that forced fp32):
      y = c0 + c1 x + c2 x^2 + c3 x^3
        + dR1 relu(x-.2)^3 + dR2 relu(x-.6)^3
        + dL1 relu(-x-.2)^3 + dL2 relu(-x-.6)^3
    (host precomputes all 8 per-edge coefs in float64, mask folded in;
    measured end-to-end rel err ~2e-3 vs the 2e-2 gate).
  - Features (xc, x^2, x^3, 4x relu^3) computed full-size in fp16:
    tensor_scalar runs 4x and tensor_tensor 2x on DVE for fp16.
  - The per-edge combine sum_p c_p[i,j]*f_p[i,j,b] runs on the TENSOR
    engine as 8 accumulating block-diagonal fp16 matmuls per j:
    lhsT = diag(c_p[:,j]) so out[i,b] += c_p[i]*f_p[i,b]. The 128
    diagonal tiles are host-built and DMAed (4 MB, overlapped).
  - mean_t(x_history) streams as fp8 through K=128 matmuls with an
    all-ones lhsT [128,128] -> PSUM rows already broadcast across all
    partitions; ACT applies sigmoid straight from PSUM with per-edge
    scale=mod_w/T and bias=mod_b.
  - x_lagged: 16 bf16 matmuls (K=11).  z = y*alpha on DVE (f32 PSUM x
    f32 alpha -> fp16), summed over j on GpSimd; fp16 partials to host.
"""

import os
import sys

import ml_dtypes
import numpy as np

for _p in ("/opt/trn_rl_repo", "/root/.axon_site/_ro/trn_rl_repo"):
    if os.path.isdir(_p) and _p not in sys.path:
        sys.path.insert(0, _p)

import concourse.bass as bass  # noqa: E402
import concourse.tile as tile  # noqa: E402
from concourse import bacc, mybir  # noqa: E402
from concourse import bass_utils  # noqa: E402

# ---------------------------------------------------------------- constants
B, T, I, O = 256, 512, 128, 128
L = 11                      # MAX_LAG + 1 lag taps
NCORES = 8
JC = I // NCORES            # j's per core = 16
GRID_SIZE, SPLINE_ORDER = 5, 3
GRID_LO, GRID_HI = -1.0, 1.0
H = (GRID_HI - GRID_LO) / GRID_SIZE
NP = 8                      # features: 1, x, x2, x3, rR1^3, rR2^3, rL1^3, rL2^3

F32 = mybir.dt.float32
F16 = mybir.dt.float16
BF16 = mybir.dt.bfloat16
FP8 = mybir.dt.float8e4
ALU = mybir.AluOpType
ACTF = mybir.ActivationFunctionType

NP_F16 = np.float16
NP_BF16 = ml_dtypes.bfloat16
NP_FP8 = ml_dtypes.float8_e4m3


# ------------------------------------------------------- host-side spline math
def _b_splines_np(x):
    """float64 copy of the reference b_splines (incl. its 1e-8 epsilons)."""
    g = (np.arange(-SPLINE_ORDER, GRID_SIZE + SPLINE_ORDER + 1, dtype=np.float64)
         * H + GRID_LO)
    x = np.asarray(x, dtype=np.float64)[..., None]
    bases = ((x >= g[:-1]) & (x < g[1:])).astype(np.float64)
    for i in range(1, SPLINE_ORDER + 1):
        t1 = (x - g[: -(i + 1)]) / (g[i:-1] - g[: -(i + 1)] + 1e-8) * bases[..., :-1]
        t2 = (g[i + 1:] - x) / (g[i + 1:] - g[1:-i] + 1e-8) * bases[..., 1:]
        bases = t1 + t2
    return bases


def _segment_poly_mats():
    """A[s] (4x8): on segment s, sum_c coef_c*B_c(x) = sum_d x^d*(A[s][d]@coef)."""
    mats = []
    for s in range(GRID_SIZE):
        lo = GRID_LO + s * H
        pts = lo + H * np.array([0.125, 0.375, 0.625, 0.875])
        Bm = _b_splines_np(pts)                       # [4, 8]
        V = np.vander(pts, 4, increasing=True)        # [4, 4]
        mats.append(np.linalg.solve(V, Bm))           # [4, 8]
    return np.stack(mats)                             # [5, 4, 8]


def _two_sided_params(coef64, mask):
    """[O, I, 8] float64: c0..c3 (center-segment cubic), dR1,dR2,dL1,dL2."""
    Am = _segment_poly_mats()                          # [5,4,8]
    a = np.einsum("sdc,oic->sdoi", Am, coef64)         # [5,4,O,I]
    p = np.empty((O, I, NP), dtype=np.float64)
    p[..., 0:4] = np.moveaxis(a[2], 0, -1)             # center cubic
    p[..., 4] = a[3, 3] - a[2, 3]                      # jump at +0.2
    p[..., 5] = a[4, 3] - a[3, 3]                      # jump at +0.6
    p[..., 6] = -(a[1, 3] - a[2, 3])                   # knot -0.2, relu(-x-.2)^3
    p[..., 7] = -(a[0, 3] - a[1, 3])                   # knot -0.6, relu(-x-.6)^3
    return p * mask[..., None]


def _host_precompute(x_history, coef, lag_logits, mod_w, mod_b, adj_logits):
    xh = np.asarray(x_history, dtype=np.float32)
    coef64 = np.asarray(coef, dtype=np.float64)
    ll = np.asarray(lag_logits, dtype=np.float64)

    m = ll.max(axis=-1, keepdims=True)
    e = np.exp(ll - m)
    w_lag = e / e.sum(axis=-1, keepdims=True)          # [O,I,L] f64

    mask = 1.0 / (1.0 + np.exp(-np.asarray(adj_logits, np.float64)[:O, :I]))
    params = _two_sided_params(coef64, mask)           # [O,I,8]

    window = xh[:, T - L:T, :][:, ::-1, :]             # [B,L,I]
    xh_tjb = np.ascontiguousarray(xh.transpose(1, 2, 0))  # [T, I, B]
    xh8_full = xh_tjb.astype(NP_FP8)

    rng = np.arange(128)
    in_maps = []
    for c in range(NCORES):
        sl = slice(c * JC, (c + 1) * JC)
        win = np.ascontiguousarray(
            window[:, :, sl].transpose(1, 2, 0)).astype(NP_BF16)   # [L,JC,B]
        wlg = np.ascontiguousarray(
            w_lag[:, sl, :].transpose(2, 1, 0)).astype(NP_BF16)    # [L,JC,O]
        xh8 = np.ascontiguousarray(xh8_full[:, sl, :])             # [T,JC,B]
        # diagonal combine tiles: [128 rows, j, p, 128 cols] (j-major so the
        # DMA can stream the tiles in j order)
        dg = np.zeros((128, JC, NP, 128), dtype=NP_F16)
        dg[rng, :, :, rng] = params[:, sl, :].transpose(0, 1, 2)   # [O,JC,NP]
        sigsc = np.ascontiguousarray(
            np.asarray(mod_w, np.float64)[:, sl] / T).astype(np.float32)
        sigbi = np.ascontiguousarray(
            np.asarray(mod_b, np.float64)[:, sl]).astype(np.float32)
        in_maps.append({
            "win": win,
            "wlag": wlg,
            "xh8": xh8,
            "diag": np.ascontiguousarray(dg.reshape(128, NP * JC * 128)),
            "ones16": np.ones((128, B), dtype=NP_F16),
            "ones8": np.ones((128, 128), dtype=NP_FP8),
            "sigsc": sigsc,
            "sigbi": sigbi,
        })
    return in_maps


# ------------------------------------------------------------- device program
def _build_program():
    nc = bacc.Bacc("TRN2", target_bir_lowering=False, debug=False,
                   num_devices=NCORES)

    win_d = nc.dram_tensor("win", [L, JC, B], BF16, kind="ExternalInput")
    wlag_d = nc.dram_tensor("wlag", [L, JC, O], BF16, kind="ExternalInput")
    xh8_d = nc.dram_tensor("xh8", [T, JC, B], FP8, kind="ExternalInput")
    diag_d = nc.dram_tensor("diag", [128, NP * JC * 128], F16,
                            kind="ExternalInput")
    ones16_d = nc.dram_tensor("ones16", [128, B], F16, kind="ExternalInput")
    ones8_d = nc.dram_tensor("ones8", [128, 128], FP8, kind="ExternalInput")
    sigsc_d = nc.dram_tensor("sigsc", [O, JC], F32, kind="ExternalInput")
    sigbi_d = nc.dram_tensor("sigbi", [O, JC], F32, kind="ExternalInput")
    out_d = nc.dram_tensor("outp", [O, B], F16, kind="ExternalOutput")

    KN = [(0, -0.2), (0, -0.6), (1, -0.2), (1, -0.6)]   # (use_negx, bias)

    with tile.TileContext(nc) as tc:
        with (
            tc.tile_pool(name="pers", bufs=1) as pers,
            tc.tile_pool(name="xhp", bufs=4) as xhp,
            tc.tile_pool(name="psm", bufs=8, space=bass.MemorySpace.PSUM) as psm,
        ):
            # ---------------- persistent loads (order = DMA priority)
            win_sb = pers.tile([L, JC, B], BF16, tag="win")
            nc.sync.dma_start(win_sb[:], win_d[:])
            wlag_sb = pers.tile([L, JC, O], BF16, tag="wlag")
            nc.sync.dma_start(wlag_sb[:], wlag_d[:])
            ones16 = pers.tile([128, B], F16, tag="ones16")
            nc.sync.dma_start(ones16[:], ones16_d[:])
            ones8 = pers.tile([128, 128], FP8, tag="ones8")
            nc.sync.dma_start(ones8[:], ones8_d[:])
            sigsc = pers.tile([O, JC], F32, tag="sigsc")
            nc.sync.dma_start(sigsc[:], sigsc_d[:])
            sigbi = pers.tile([O, JC], F32, tag="sigbi")
            nc.sync.dma_start(sigbi[:], sigbi_d[:])

            # fp8 history: 8 tiles, each packing the 4 t-chunks of a j-pair
            xh_view = xh8_d.rearrange("(c p) j b -> p c (j b)", c=4)
            xts = []
            for r in range(8):
                xt = xhp.tile([128, 4, 512], FP8, tag="xh", name=f"xh{r}")
                nc.sync.dma_start(xt[:], xh_view[:, :, r * 512:(r + 1) * 512])
                xts.append(xt)

            # diagonal coef tiles, streamed in j order
            diag = pers.tile([128, JC * NP * 128], F16, tag="diag")
            DGCH = NP * 128
            for j in range(JC):
                nc.sync.dma_start(diag[:, j * DGCH:(j + 1) * DGCH],
                                  diag_d[:, j * DGCH:(j + 1) * DGCH])

            def dg(p, j):
                off = (j * NP + p) * 128
                return diag[:, off:off + 128]

            # ---------------- PE: x_lagged (bf16, K=11) -> 8 psum tiles
            ps_xl = []
            for t in range(8):                      # tile t covers j = 2t, 2t+1
                pt = psm.tile([128, 2 * B], F32, tag="ps", name=f"xl{t}")
                ps_xl.append(pt)
                for h in range(2):
                    jl = 2 * t + h
                    nc.tensor.matmul(pt[:, h * B:(h + 1) * B],
                                     wlag_sb[:, jl, :], win_sb[:, jl, :],
                                     start=True, stop=True)

            # ---------------- PE: mean stream (fp8, rows broadcast)
            pms = []
            for r in range(8):
                pm = psm.tile([128, 512], F32, tag="ps", name=f"pm{r}")
                pms.append(pm)
                for ch in range(4):
                    nc.tensor.matmul(pm[:], ones8[:], xts[r][:, ch, :],
                                     start=(ch == 0), stop=(ch == 3))

            # ---------------- ACT: sigmoids first (release mean banks)
            alpha = pers.tile([128, JC * B], F32, tag="alpha")
            for r in range(8):
                for h in range(2):
                    jl = 2 * r + h
                    nc.scalar.activation(
                        alpha[:, jl * B:(jl + 1) * B],
                        pms[r][:, h * B:(h + 1) * B],
                        ACTF.Sigmoid, bias=sigbi[:, jl:jl + 1],
                        scale=sigsc[:, jl:jl + 1])

            # ---------------- DVE: clip -> xc fp16 (1x from PSUM)
            xc = pers.tile([128, JC * B], F16, tag="xc")
            for t in range(8):
                nc.vector.tensor_scalar(xc[:, t * 512:(t + 1) * 512],
                                        ps_xl[t][:], -1.0, 1.0,
                                        op0=ALU.max, op1=ALU.min)

            # ---------------- features (fp16); ts = 4x, tt = 2x on DVE
            negx = pers.tile([128, JC * B], F16, tag="negx")
            nc.vector.tensor_scalar(negx[:], xc[:], -1.0, None, op0=ALU.mult)
            r_t = []
            for k, (useneg, bia) in enumerate(KN):
                r = pers.tile([128, JC * B], F16, tag=f"r{k}", name=f"r{k}")
                nc.vector.tensor_scalar(r[:], (negx if useneg else xc)[:],
                                        bia, 0.0, op0=ALU.add, op1=ALU.max)
                r_t.append(r)
            # squares: knot 0 on ACT (after sigmoids), 1..3 on DVE; x2 on ACT
            x2 = pers.tile([128, JC * B], F16, tag="x2")
            nc.scalar.activation(x2[:], xc[:], ACTF.Square)
            q_t = [pers.tile([128, JC * B], F16, tag=f"q{k}", name=f"q{k}")
                   for k in range(4)]
            nc.scalar.activation(q_t[0][:], r_t[0][:], ACTF.Square)
            for k in (1, 2, 3):
                nc.vector.tensor_tensor(q_t[k][:], r_t[k][:], r_t[k][:],
                                        op=ALU.mult)
            r3_t = [pers.tile([128, JC * B], F16, tag=f"c{k}", name=f"c{k}")
                    for k in range(4)]
            for k in (1, 2, 3):
                nc.vector.tensor_tensor(r3_t[k][:], q_t[k][:], r_t[k][:],
                                        op=ALU.mult)
            x3 = pers.tile([128, JC * B], F16, tag="x3")
            nc.vector.tensor_tensor(x3[:], x2[:], xc[:], op=ALU.mult)
            nc.vector.tensor_tensor(r3_t[0][:], q_t[0][:], r_t[0][:],
                                    op=ALU.mult)

            # ---------------- PE: combine as feature sweeps over 8 y-tiles
            # feats index -> coef slot p: 0:ones 1:xc 2:x2 3:x3 4..7:r3_k
            feats = [None, xc, x2, x3] + r3_t
            yts = [psm.tile([128, 512], F32, tag="ps", name=f"y{t}")
                   for t in range(8)]

            def comb_mm(p, jl, start, stop):
                rhs = (ones16[:] if p == 0
                       else feats[p][:, jl * B:(jl + 1) * B])
                nc.tensor.matmul(yts[jl // 2][:, (jl % 2) * B:(jl % 2 + 1) * B],
                                 dg(p, jl), rhs, start=start, stop=stop)

            zb = pers.tile([128, JC * B], F16, tag="zb")
            acc = pers.tile([128, B], F16, tag="acc")
            for t in range(8):                      # per-j closed groups
                for jl in (2 * t, 2 * t + 1):
                    for pi, p in enumerate((0, 1, 2, 3, 5, 7, 4, 6)):
                        comb_mm(p, jl, start=(pi == 0), stop=(pi == 7))
                # z = y * alpha for the pair (f32 PSUM x f32 -> fp16)
                nc.vector.tensor_tensor(zb[:, t * 512:(t + 1) * 512],
                                        yts[t][:],
                                        alpha[:, t * 512:(t + 1) * 512],
                                        op=ALU.mult)
                # j-sum on GpSimd, incremental
                if t == 0:
                    nc.gpsimd.tensor_tensor(acc[:], zb[:, 0:B], zb[:, B:2 * B],
                                            op=ALU.add)
                else:
                    nc.gpsimd.tensor_tensor(acc[:], acc[:],
                                            zb[:, t * 512:t * 512 + B],
                                            op=ALU.add)
                    nc.gpsimd.tensor_tensor(acc[:], acc[:],
                                            zb[:, t * 512 + B:(t + 1) * 512],
                                            op=ALU.add)

            nc.sync.dma_start(out_d[:], acc[:])

    nc.compile()
    return nc


_CACHED_NC = None


def _get_program():
    global _CACHED_NC
    if _CACHED_NC is None:
        _CACHED_NC = _build_program()
    return _CACHED_NC


# ------------------------------------------------------------------ entry
def kernel(x_history, coef, lag_logits, mod_w, mod_b, adj_logits):
    in_maps = _host_precompute(x_history, coef, lag_logits, mod_w, mod_b,
                               adj_logits)
    nc = _get_program()
    res = bass_utils.run_bass_kernel_spmd(nc, in_maps,
                                          core_ids=list(range(NCORES)))
    total = np.zeros((O, B), dtype=np.float64)
    for c in range(NCORES):
        total += np.asarray(res.results[c]["outp"], dtype=np.float64)
    return np.ascontiguousarray(total.T.astype(np.float32))


# -------------------------------------------- pure-numpy emulation (testing)
def emulate(x_history, coef, lag_logits, mod_w, mod_b, adj_logits):
    """Numpy mirror of the v2 device algorithm (f32-ish, no dtype sim)."""
    in_maps = _host_precompute(x_history, coef, lag_logits, mod_w, mod_b,
                               adj_logits)
    total = np.zeros((O, B), dtype=np.float64)
    for c in range(NCORES):
        total += emulate_core(in_maps[c])
    return total.T.astype(np.float32)


def emulate_core(im):
    win = im["win"].astype(np.float64)            # [L,JC,B]
    wlg = im["wlag"].astype(np.float64)           # [L,JC,O]
    dgf = im["diag"].astype(np.float64).reshape(128, JC, NP, 128)
    params = dgf[np.arange(128), :, :, np.arange(128)]   # [128,JC,NP] (o,j,p)
    params = params.transpose(0, 2, 1)                   # [128,NP,JC]
    xm = im["xh8"].astype(np.float64).mean(axis=0)       # [JC,B]
    sigsc = im["sigsc"].astype(np.float64)        # [O,JC]
    sigbi = im["sigbi"].astype(np.float64)

    part = np.zeros((O, B), dtype=np.float64)
    for jl in range(JC):
        xl = wlg[:, jl, :].T @ win[:, jl, :]      # [O,B]
        x = np.clip(xl, -1.0, 1.0)
        f = [np.ones_like(x), x, x * x, x ** 3,
             np.maximum(x - 0.2, 0) ** 3, np.maximum(x - 0.6, 0) ** 3,
             np.maximum(-x - 0.2, 0) ** 3, np.maximum(-x - 0.6, 0) ** 3]
        y = np.zeros_like(x)
        for p in range(NP):
            y += params[:, p, jl][:, None] * f[p]
        lin = sigsc[:, jl][:, None] * (xm[jl] * T)[None, :] + sigbi[:, jl][:, None]
        part += y / (1.0 + np.exp(-lin))
    return part
